# revision 24
# baseline (speedup 1.0000x reference)
"""Trainium2 Bass kernel for the DEAM dense-CNN block.

Data-parallel over batch: 16 samples -> 8 cores x 2 samples.
Per sample: attention chain (GAP -> conv1d -> sigmoid/softmax heads),
dynamic per-sample 3x3 conv as 9 shifted matmuls, LGA gating branch,
fused add + batch BN (cross-core AllReduce of per-channel sums) + ReLU.

Channel shuffle is folded into host-side weight permutations plus a
permuted output DMA, so no on-chip data movement is spent on it.

v7 (trace-driven; lineage 294us -> 275 -> 254 -> ~this):
- chain split: softmax/kia path (part a) emitted and scheduled ahead
  of the LGA mlp path (part b); the 4.4us max-reduces are split in
  half and dependency-pinned off the critical path.
- GAP sums: tile-0 on ACT (Copy+accum_out) parallel with tile-1 on
  DVE; full-width dense reads (zero padding keeps sums exact, channel
  maxes verified positive).
- agg weight combine via 4x-mode tensor_scalar + 2x-mode tensor_tensor.
- BN: 2 stat AllReduces (ct0 mid-conv fully hidden; one final ct1) +
  a start-of-kernel alignment AllReduce; stat DMAs on gpsimd ring,
  readbacks on sync ring.
- z tiles and output bf16 (host upcasts); finalize full 128-partition
  tiles split across ACT and DVE.
"""

import numpy as np
import ml_dtypes

import concourse.bass as bass
import concourse.mybir as mybir
import concourse.tile as tile
from concourse import bacc
from concourse.bass_utils import run_bass_kernel_spmd
from concourse.masks import make_identity

F32 = mybir.dt.float32
BF16 = mybir.dt.bfloat16
AX = mybir.AxisListType
ALU = mybir.AluOpType
ACT = mybir.ActivationFunctionType

B, C, H, W = 16, 256, 64, 64
HW = H * W
KNUM, KS = 4, 3
N_CORES = 8
B_LOC = B // N_CORES          # 2 samples per core
NT = C // 128                 # 2 channel tiles
BN_EPS = 1e-5
XW = W + 2                    # padded row width 66
PQ = KS * KS                  # 9
EFREE = PQ * C                # 2304 free elems of an agg/E tile
EHALF = EFREE // 2            # 1152

REPLICAS = [list(range(N_CORES))]

# shift order: (0,0) first so the start=True matmul covers the full bank
SHIFTS = [(0, 0), (0, -1), (0, 1), (-1, -1), (-1, 0), (-1, 1),
          (1, -1), (1, 0), (1, 1)]


def _pin(inst, after, why):
    if after is not None:
        bass._add_dep_helper(inst.ins, after.ins, sync=False, reason=why)
    return inst


def build_program():
    cdt = BF16
    edt = BF16

    nc = bacc.Bacc("TRN2", target_bir_lowering=False, debug=False,
                   num_devices=N_CORES)

    x_d = nc.dram_tensor("x", [B_LOC, C, H, XW], cdt, kind="ExternalInput")
    e_d = nc.dram_tensor("ew", [KNUM, NT, 128, EFREE], edt,
                         kind="ExternalInput")
    aow_d = nc.dram_tensor("aow", [NT, 128, C], F32, kind="ExternalInput")
    akw_d = nc.dram_tensor("akw", [NT, 128, KNUM], F32, kind="ExternalInput")
    w1t_d = nc.dram_tensor("w1t", [128, 16], F32, kind="ExternalInput")
    w2t_d = nc.dram_tensor("w2t", [16, 128], F32, kind="ExternalInput")
    gb_d = nc.dram_tensor("gb", [NT, 128, 2], F32, kind="ExternalInput")
    sm_d = nc.dram_tensor("sm", [1, 8], F32, kind="ExternalInput")
    out_d = nc.dram_tensor("out", [B_LOC, C, H, W], BF16,
                           kind="ExternalOutput")
    dbg_d = nc.dram_tensor("dbg", [1, 8], F32, kind="ExternalOutput")

    with tile.TileContext(nc) as tc:
        with (
            tc.tile_pool(name="singles", bufs=1) as singles,
            tc.tile_pool(name="xq", bufs=1) as xq_pool,
            tc.tile_pool(name="z", bufs=1) as z_pool,
            tc.tile_pool(name="ep", bufs=16) as e_pool,
            tc.tile_pool(name="aggb", bufs=1) as aggb_pool,
            tc.tile_pool(name="atmp", bufs=2) as atmp_pool,
            tc.tile_pool(name="sp", bufs=2) as sp,
            tc.tile_pool(name="ob", bufs=2) as ob_pool,
            tc.tile_pool(name="psc", bufs=6, space="PSUM") as ps_conv,
            tc.tile_pool(name="pss", bufs=2, space="PSUM") as pss,
            tc.tile_pool(name="dram", bufs=8, space="DRAM") as dram,
        ):
            # ---- alignment AllReduce: absorbs cross-core launch skew ----
            alz = singles.tile([1, 8], F32, tag="alz")
            nc.vector.memset(alz[:, :], 0.0)
            ali = dram.tile([1, 8], F32, tag="ali", name="ali", bufs=1)
            alo = dram.tile([1, 8], F32, tag="alo", name="alo", bufs=1)
            nc.gpsimd.dma_start(out=ali[:, :], in_=alz[:, :])
            nc.gpsimd.collective_compute(
                "AllReduce", ALU.add, replica_groups=REPLICAS,
                ins=[ali[:, :].opt()], outs=[alo[:, :].opt()])
            albk = singles.tile([1, 8], F32, tag="albk")
            nc.gpsimd.dma_start(out=albk[:, :], in_=alo[:, :])
            nc.sync.dma_start(out=dbg_d[:, :], in_=albk[:, :])

            # ---- x[0] first (chain-0 gates everything) ----
            xq = [[None] * NT for _ in range(B_LOC)]
            for t in range(NT):
                xt = xq_pool.tile([128, H, XW], cdt, tag=f"xq0{t}",
                                  name=f"xq0{t}", bufs=1)
                nc.sync.dma_start(out=xt[:, :, :],
                                  in_=x_d[0, t * 128:(t + 1) * 128])
                xq[0][t] = xt

            # ---- constants ----
            ident = singles.tile([128, 128], F32, tag="ident")
            make_identity(nc, ident[:, :])
            smalls = singles.tile([1, 8], F32, tag="smalls")
            nc.sync.dma_start(out=smalls[:, :], in_=sm_d[:, :])
            aow_s = []
            akw_s = []
            for t in range(NT):
                a = singles.tile([128, C], F32, tag=f"aow{t}")
                nc.sync.dma_start(out=a[:, :], in_=aow_d[t])
                aow_s.append(a)
                k = singles.tile([128, KNUM], F32, tag=f"akw{t}")
                nc.sync.dma_start(out=k[:, :], in_=akw_d[t])
                akw_s.append(k)
            w1t_s = singles.tile([128, 16], F32, tag="w1t")
            nc.sync.dma_start(out=w1t_s[:, :], in_=w1t_d[:, :])
            w2t_s = singles.tile([16, 128], F32, tag="w2t")
            nc.sync.dma_start(out=w2t_s[:, :], in_=w2t_d[:, :])
            gb_s = singles.tile([128, NT, 2], F32, tag="gb")
            for t in range(NT):
                nc.sync.dma_start(out=gb_s[:, t, :], in_=gb_d[t])
            eps_t = singles.tile([128, 1], F32, tag="eps_t")
            nc.vector.memset(eps_t[:, :], BN_EPS)
            ones1 = singles.tile([1, 128], F32, tag="ones1")
            nc.vector.memset(ones1[:, :], 1.0)

            # per-ct batch-norm stat chunks (both samples) + AR results
            st = [singles.tile([128, 16, 6], F32, tag=f"st{t}",
                               name=f"st{t}") for t in range(NT)]
            sg0 = singles.tile([128, 2], F32, tag="sg0")
            sg1 = singles.tile([128, 2], F32, tag="sg1")
            ascr = [singles.tile([128, H * XW], BF16, tag=f"ascr{i}",
                                 name=f"ascr{i}") for i in range(2)]

            # ---- E weights, then x[1] ----
            e_tiles = {}
            for t in range(NT):
                for k in range(KNUM):
                    for hh in range(2):
                        et = e_pool.tile([128, EHALF], edt, tag="e")
                        nc.sync.dma_start(
                            out=et[:, :],
                            in_=e_d[k, t, :, hh * EHALF:(hh + 1) * EHALF])
                        e_tiles[(t, k, hh)] = et

            for t in range(NT):
                xt = xq_pool.tile([128, H, XW], cdt, tag=f"xq1{t}",
                                  name=f"xq1{t}", bufs=1)
                nc.sync.dma_start(out=xt[:, :, :],
                                  in_=x_d[1, t * 128:(t + 1) * 128])
                xq[1][t] = xt

            # ---- agg + z allocation ----
            aggb = [[None] * NT for _ in range(B_LOC)]
            for i in range(B_LOC):
                for t in range(NT):
                    aggb[i][t] = aggb_pool.tile(
                        [128, EFREE], cdt, tag=f"aggb{i}{t}",
                        name=f"aggb{i}{t}", bufs=1)
            z = [[None] * NT for _ in range(B_LOC)]
            for i in range(B_LOC):
                for t in range(NT):
                    z[i][t] = z_pool.tile([128, HW], BF16, tag=f"z{i}{t}",
                                          name=f"z{i}{t}", bufs=1)

            # ---- chain part a: GAP -> conv1d -> sigmoid/softmax -> kia ----
            def emit_chain_a(i, dve_after=None, act_after=None):
                ch = {}
                gsum = sp.tile([128, NT], F32, tag="gsum",
                               name=f"gsum{i}", bufs=1)
                _pin(nc.scalar.activation(
                    out=ascr[i][:, :],
                    in_=xq[i][0][:, :, :].rearrange("p h w -> p (h w)"),
                    func=ACT.Copy, accum_out=gsum[:, 0:1]),
                    act_after, "chain act-sum ordered")
                _pin(nc.vector.tensor_reduce(
                    gsum[:, 1:2],
                    xq[i][1][:, :, :].rearrange("p h w -> p (h w)"),
                    axis=AX.X, op=ALU.add), dve_after, "chain sum ordered")

                # gap -> free layout (two (128,1) -> (1,128) transposes)
                gf = []
                for t in range(NT):
                    g_ps = pss.tile([1, 128], F32, tag="pst")
                    nc.tensor.transpose(g_ps[:, :], gsum[:, t:t + 1],
                                        ident[:, :])
                    gf.append(g_ps)
                g2 = sp.tile([1, C + 2], F32, tag="g2")
                nc.vector.memset(g2[:, :], 0.0)
                for t in range(NT):
                    nc.vector.tensor_copy(
                        out=g2[0:1, 1 + t * 128:1 + (t + 1) * 128],
                        in_=gf[t][0:1, :])
                gs = sp.tile([1, 130], F32, tag="gs")
                nc.vector.memset(gs[:, :], 0.0)
                nc.vector.tensor_copy(out=gs[0:1, 1:129], in_=gf[0][0:1, :])

                # t = conv1d(gap_mean, att_w) : weights pre-scaled by 1/HW
                ta = sp.tile([1, C], F32, tag="ta")
                tb = sp.tile([1, C], F32, tag="tb")
                t_t = sp.tile([1, C], F32, tag="t_t")
                nc.vector.tensor_scalar_mul(ta[:, :], g2[0:1, 0:C],
                                            smalls[0:1, 0:1])
                nc.vector.scalar_tensor_tensor(tb[:, :], g2[0:1, 1:C + 1],
                                               smalls[0:1, 1:2], ta[:, :],
                                               ALU.mult, ALU.add)
                nc.vector.scalar_tensor_tensor(t_t[:, :], g2[0:1, 2:C + 2],
                                               smalls[0:1, 2:3], tb[:, :],
                                               ALU.mult, ALU.add)

                # s = conv1d(gap1_mean, lga_w) + b (bias in free layout)
                sa_ = sp.tile([1, 128], F32, tag="sa_")
                sb_ = sp.tile([1, 128], F32, tag="sb_")
                s_t = sp.tile([1, 128], F32, tag="s_t")
                nc.vector.tensor_scalar_mul(sa_[:, :], gs[0:1, 0:128],
                                            smalls[0:1, 3:4])
                nc.vector.scalar_tensor_tensor(sb_[:, :], gs[0:1, 1:129],
                                               smalls[0:1, 4:5], sa_[:, :],
                                               ALU.mult, ALU.add)
                nc.vector.scalar_tensor_tensor(s_t[:, :], gs[0:1, 2:130],
                                               smalls[0:1, 5:6], sb_[:, :],
                                               ALU.mult, ALU.add)
                nc.vector.tensor_scalar_add(s_t[:, :], s_t[:, :],
                                            smalls[0:1, 6:7])

                # transposes back to partition layout
                tps = sp.tile([128, NT], F32, tag="tps")
                ia = sp.tile([128, NT], F32, tag="ia")
                for t in range(NT):
                    tp_ps = pss.tile([128, 1], F32, tag="pst")
                    nc.tensor.transpose(tp_ps[:, :],
                                        t_t[0:1, t * 128:(t + 1) * 128],
                                        ident[0:1, 0:1])
                    nc.vector.tensor_copy(out=tps[:, t:t + 1],
                                          in_=tp_ps[:, :])
                    nc.scalar.activation(out=ia[:, t:t + 1], in_=tp_ps[:, :],
                                         func=ACT.Sigmoid)
                sk = sp.tile([128, 2], F32, tag="sk", name=f"sk{i}", bufs=1)
                sp_ps = pss.tile([128, 1], F32, tag="pst")
                nc.tensor.transpose(sp_ps[:, :], s_t[0:1, :],
                                    ident[0:1, 0:1])
                nc.scalar.activation(out=sk[:, 0:1], in_=sp_ps[:, :],
                                     func=ACT.Sigmoid)

                # kernel attention logits -> softmax (critical path to agg)
                kl_ps = pss.tile([KNUM, 1], F32, tag="pst")
                for t in range(NT):
                    nc.tensor.matmul(kl_ps[:, :], akw_s[t][:, :],
                                     tps[:, t:t + 1],
                                     start=(t == 0), stop=(t == NT - 1))
                kls = sp.tile([KNUM, 1], F32, tag="kls")
                nc.vector.tensor_copy(out=kls[:, :], in_=kl_ps[:, :])
                kt_ps = pss.tile([1, KNUM], F32, tag="pst")
                nc.tensor.transpose(kt_ps[:, :], kls[:, :],
                                    ident[0:KNUM, 0:KNUM])
                mx = sp.tile([1, 1], F32, tag="mx")
                nc.vector.reduce_max(mx[:, :], kt_ps[0:1, :], axis=AX.X)
                ex = sp.tile([1, KNUM], F32, tag="ex")
                nc.vector.tensor_scalar(out=ex[:, :], in0=kt_ps[0:1, :],
                                        scalar1=mx[:, :], scalar2=None,
                                        op0=ALU.subtract)
                exs = sp.tile([1, KNUM], F32, tag="exs")
                ch["exp_inst"] = nc.scalar.activation(out=exs[:, :],
                                                      in_=ex[:, :],
                                                      func=ACT.Exp)
                sm1 = sp.tile([1, 1], F32, tag="sm1")
                nc.vector.reduce_sum(sm1[:, :], exs[:, :], axis=AX.X)
                nc.vector.reciprocal(out=sm1[:, :], in_=sm1[:, :])
                katt = sp.tile([1, KNUM], F32, tag="katt")
                nc.vector.tensor_scalar_mul(katt[:, :], exs[:, :],
                                            sm1[:, :])
                kattb = sp.tile([128, KNUM], F32, tag="kattb")
                kb_ps = pss.tile([128, KNUM], F32, tag="pst")
                nc.tensor.matmul(kb_ps[:, :], ones1[:, :], katt[0:1, :],
                                 start=True, stop=True)
                nc.vector.tensor_copy(out=kattb[:, :], in_=kb_ps[:, :])

                # fold in_att into softmax scalars: kia[ci,k] = katt_k*ia[ci]
                kia = sp.tile([128, NT, KNUM], F32, tag="kia")
                kia_last = None
                for t in range(NT):
                    kia_last = nc.vector.tensor_scalar_mul(
                        kia[:, t, :], kattb[:, :], ia[:, t:t + 1])
                ch["kia_last"] = kia_last

                # out_att (permuted) per co tile -- needed only at drains
                oatt = sp.tile([128, NT], F32, tag="oatt",
                               name=f"oatt{i}", bufs=1)
                for ct in range(NT):
                    o_ps = pss.tile([128, 1], F32, tag="pst")
                    for t in range(NT):
                        nc.tensor.matmul(
                            o_ps[:, :],
                            aow_s[t][:, ct * 128:(ct + 1) * 128],
                            tps[:, t:t + 1],
                            start=(t == 0), stop=(t == NT - 1))
                    nc.scalar.activation(out=oatt[:, ct:ct + 1],
                                         in_=o_ps[:, :], func=ACT.Sigmoid)

                ch["kia"] = kia
                ch["oatt"] = oatt
                ch["sk"] = sk
                ch["gsum"] = gsum
                return ch

            # ---- chain part b: LGA mlp (max/mean -> mlp -> sk[1]) ----
            def emit_chain_b(i, ch, dve_after=None):
                vv = sp.tile([128, 2], F32, tag="vv")
                xf1 = xq[i][1][:, :, :].rearrange("p h w -> p (h w)")
                hx = sp.tile([128, 2], F32, tag="hx")
                _pin(nc.vector.tensor_reduce(
                    hx[:, 0:1], xf1[:, 0:H * XW // 2],
                    axis=AX.X, op=ALU.max), dve_after, "max half ordered")
                _pin(nc.vector.tensor_reduce(
                    hx[:, 1:2], xf1[:, H * XW // 2:],
                    axis=AX.X, op=ALU.max), dve_after, "max half ordered")
                nc.vector.tensor_reduce(vv[:, 0:1], hx[:, :],
                                        axis=AX.X, op=ALU.max)
                nc.vector.tensor_scalar_mul(vv[:, 1:2],
                                            ch["gsum"][:, 1:2], 1.0 / HW)
                h_ps = pss.tile([16, 2], F32, tag="pst")
                nc.tensor.matmul(h_ps[:, :], w1t_s[:, :], vv[:, :],
                                 start=True, stop=True)
                h_s = sp.tile([16, 2], F32, tag="h_s")
                nc.vector.tensor_scalar_max(h_s[:, :], h_ps[:, :], 0.0)
                m_ps = pss.tile([128, 2], F32, tag="pst")
                nc.tensor.matmul(m_ps[:, :], w2t_s[:, :], h_s[:, :],
                                 start=True, stop=True)
                mcp = sp.tile([128, 2], F32, tag="mcp")
                nc.vector.tensor_copy(out=mcp[:, :], in_=m_ps[:, :])
                chadd = sp.tile([128, 1], F32, tag="chadd")
                nc.vector.tensor_add(chadd[:, :], mcp[:, 0:1], mcp[:, 1:2])
                nc.scalar.activation(out=ch["sk"][:, 1:2], in_=chadd[:, :],
                                     func=ACT.Sigmoid)

            def emit_agg(i, ch):
                # dst = sum_k kia_k * E_k, via 4x-mode TS + 2x-mode TT add
                last = None
                for t in range(NT):
                    for hh in range(2):
                        dst = aggb[i][t][:, hh * EHALF:(hh + 1) * EHALF]
                        kap0 = ch["kia"][:, t, 0:1]
                        nc.vector.tensor_scalar_mul(dst, e_tiles[(t, 0, hh)],
                                                    kap0)
                        for k in range(1, KNUM):
                            tmp = atmp_pool.tile([128, EHALF], BF16,
                                                 tag="atmp")
                            nc.vector.tensor_scalar_mul(
                                tmp[:, :], e_tiles[(t, k, hh)],
                                ch["kia"][:, t, k:k + 1])
                            last = nc.vector.tensor_add(dst, dst, tmp[:, :])
                return last

            def emit_zinit(i, t, ch, after=None):
                return _pin(nc.vector.tensor_scalar_mul(
                    z[i][t][:, :], xq[i][t][:, :, 1:W + 1],
                    ch["sk"][:, t:t + 1]), after, "zinit ordered")

            def emit_conv(i, ct, ch):
                for grp in range(2):
                    banks = []
                    for j in range(4):
                        banks.append(ps_conv.tile([128, 8, W], F32,
                                                  tag="cps",
                                                  name=f"cps{j}"))
                    for cit in range(NT):
                        for (dp, dq) in SHIFTS:
                            pq = (dp + 1) * 3 + (dq + 1)
                            lo = pq * C + ct * 128
                            lhs = aggb[i][cit][:, lo:lo + 128]
                            for j in range(4):
                                chunk = grp * 4 + j
                                y0 = chunk * 8
                                ylo = max(y0, -dp)
                                yhi = min(y0 + 7, H - 1 - dp)
                                n_r = yhi - ylo + 1
                                if n_r <= 0:
                                    continue
                                out_ap = banks[j][:, ylo - y0:
                                                  ylo - y0 + n_r, :]
                                in_ap = xq[i][cit][:, ylo + dp:
                                                   ylo + dp + n_r,
                                                   1 + dq:1 + dq + W]
                                first = (cit == 0 and dp == 0 and dq == 0)
                                last = (cit == NT - 1
                                        and (dp, dq) == SHIFTS[-1])
                                nc.tensor.matmul(out_ap, lhs, in_ap,
                                                 start=first, stop=last,
                                                 skip_group_check=True)
                    # drain: z = psum * out_att + z ; then partial BN stats
                    for j in range(4):
                        chunk = grp * 4 + j
                        zsl = z[i][ct][:, chunk * 512:(chunk + 1) * 512]
                        nc.vector.scalar_tensor_tensor(
                            zsl, banks[j][:, :, :],
                            ch["oatt"][:, ct:ct + 1], zsl,
                            ALU.mult, ALU.add)
                        nc.vector.bn_stats(
                            out=st[ct][:, i * 8 + grp * 4 + j, :], in_=zsl)

            def emit_ar(name, sg_tile, ct):
                mv = sp.tile([128, 2], F32, tag="mv")
                nc.vector.bn_aggr(out=mv[:, :], in_=st[ct][:, :, :])
                m2 = sp.tile([128, 1], F32, tag="m2")
                nc.vector.tensor_mul(m2[:, :], mv[:, 0:1], mv[:, 0:1])
                sa = sp.tile([128, 2], F32, tag=f"sa{name}",
                             name=f"sa{name}")
                nc.vector.tensor_scalar_mul(sa[:, 0:1], mv[:, 0:1],
                                            float(B_LOC * HW))
                ex2 = sp.tile([128, 1], F32, tag="ex2")
                nc.vector.tensor_add(ex2[:, :], mv[:, 1:2], m2[:, :])
                nc.vector.tensor_scalar_mul(sa[:, 1:2], ex2[:, :],
                                            float(B_LOC * HW))
                si = dram.tile([128, 2], F32, tag=f"si{name}",
                               name=f"si{name}", bufs=1)
                so = dram.tile([128, 2], F32, tag=f"so{name}",
                               name=f"so{name}", bufs=1)
                nc.sync.dma_start(out=si[:, :], in_=sa[:, :])
                nc.gpsimd.collective_compute(
                    "AllReduce", ALU.add, replica_groups=REPLICAS,
                    ins=[si[:, :].opt()], outs=[so[:, :].opt()])
                nc.sync.dma_start(out=sg_tile[:, :], in_=so[:, :])

            out_view = out_d[:, :, :, :].rearrange(
                "b (cl cr) h w -> b cr cl (h w)", cr=4)
            n_total = float(B * HW)

            def emit_fin(t, sg_tile):
                tot = sp.tile([128, 2], F32, tag="tot")
                nc.vector.tensor_scalar_mul(tot[:, :], sg_tile[:, 0:2],
                                            1.0 / n_total)
                mean = tot[:, 0:1]
                ex2g = tot[:, 1:2]
                m2g = sp.tile([128, 1], F32, tag="m2g")
                nc.vector.tensor_mul(m2g[:, :], mean, mean)
                var = sp.tile([128, 1], F32, tag="var")
                nc.vector.tensor_sub(var[:, :], ex2g, m2g[:, :])
                rstd = sp.tile([128, 1], F32, tag="rstd")
                nc.scalar.activation(out=rstd[:, :], in_=var[:, :],
                                     func=ACT.Sqrt, bias=eps_t[:, :])
                nc.vector.reciprocal(out=rstd[:, :], in_=rstd[:, :])
                scl = sp.tile([128, 1], F32, tag="scl")
                nc.vector.tensor_mul(scl[:, :], gb_s[:, t, 0:1], rstd[:, :])
                tmpb = sp.tile([128, 1], F32, tag="tmpb")
                nc.vector.tensor_mul(tmpb[:, :], mean, scl[:, :])
                bia = sp.tile([128, 1], F32, tag="bia")
                nc.vector.tensor_sub(bia[:, :], gb_s[:, t, 1:2], tmpb[:, :])
                for i in range(B_LOC):
                    ob = ob_pool.tile([128, HW], BF16, tag="ob")
                    if i == 0:
                        nc.scalar.activation(out=ob[:, :], in_=z[i][t][:, :],
                                             func=ACT.Relu, bias=bia[:, :],
                                             scale=scl[:, :])
                    else:
                        nc.vector.tensor_scalar(
                            out=ob[:, :], in0=z[i][t][:, :],
                            scalar1=scl[:, :], scalar2=bia[:, :],
                            op0=ALU.mult, op1=ALU.add)
                        nc.vector.tensor_scalar_max(ob[:, :], ob[:, :], 0.0)
                    for ph in range(2):
                        nc.sync.dma_start(
                            out=out_view[i, 2 * t + ph, :, :],
                            in_=ob[ph * 64:(ph + 1) * 64, :])

            # ---- program flow ----
            ch0 = emit_chain_a(0)
            agg0_last = emit_agg(0, ch0)
            zi00 = emit_zinit(0, 0, ch0, after=ch0["kia_last"])
            emit_chain_b(0, ch0, dve_after=agg0_last)
            zi01 = emit_zinit(0, 1, ch0)
            emit_conv(0, 0, ch0)
            ch1 = emit_chain_a(1, dve_after=zi01,
                               act_after=ch0["exp_inst"])
            emit_conv(0, 1, ch0)
            agg1_last = emit_agg(1, ch1)
            zi10 = emit_zinit(1, 0, ch1)
            emit_conv(1, 0, ch1)
            emit_chain_b(1, ch1, dve_after=agg1_last)
            emit_zinit(1, 1, ch1)
            emit_ar("0", sg0, 0)
            emit_conv(1, 1, ch1)
            emit_ar("1", sg1, 1)
            emit_fin(0, sg0)
            emit_fin(1, sg1)
    nc.finalize()
    return nc


def _host_prep(inputs):
    """Numpy-side weight re-layouts (all small except ede transpose)."""
    c = np.arange(C)
    pinv = (c % 64) * 4 + c // 64          # output-channel permutation
    ede = np.ascontiguousarray(inputs["ede_weight"], dtype=np.float32)
    ede_p = ede[:, pinv]                    # permute co axis
    # -> [k, ci, pq, co] so an SBUF agg tile is [ci_part, pq*256+co]
    e_host = np.ascontiguousarray(
        ede_p.transpose(0, 2, 3, 4, 1).reshape(KNUM, NT, 128, EFREE))
    e_host = e_host.astype(ml_dtypes.bfloat16)
    aow = np.ascontiguousarray(
        inputs["att_out_w"][pinv].T.reshape(NT, 128, C), dtype=np.float32)
    akw = np.ascontiguousarray(
        inputs["att_kernel_w"].T.reshape(NT, 128, KNUM), dtype=np.float32)
    w1t = np.ascontiguousarray(inputs["lga_mlp_w1"].T, dtype=np.float32)
    w2t = np.ascontiguousarray(inputs["lga_mlp_w2"].T, dtype=np.float32)
    gb = np.stack([np.asarray(inputs["bn_gamma"])[pinv].reshape(NT, 128),
                   np.asarray(inputs["bn_beta"])[pinv].reshape(NT, 128)],
                  axis=-1).astype(np.float32)
    aw = np.asarray(inputs["att_conv1d_w"], dtype=np.float32) / HW
    lw = np.asarray(inputs["lga_conv1d_w"], dtype=np.float32) / HW
    lb = float(np.asarray(inputs["lga_conv1d_b"]).reshape(-1)[0])
    sm = np.array([[aw[0], aw[1], aw[2], lw[0], lw[1], lw[2], lb, 0.0]],
                  dtype=np.float32)
    return e_host, aow, akw, w1t, w2t, gb, sm


_CACHE = {}
last_results = None


def _enable_axon_trace():
    """Register the NTFF profile hook that the agent image leaves out."""
    import sys
    import types

    import concourse.bass_utils as bu
    if "antenv.axon_hooks" in sys.modules:
        return
    from trn_agent_boot.trn_boot import _ntff_profile_via_ctypes
    hook = _ntff_profile_via_ctypes("/opt/axon/libaxon_pjrt.so")
    mod = types.ModuleType("antenv.axon_hooks")
    mod.get_axon_ntff_profile_hook = lambda: hook
    mod.set_axon_ntff_profile_hook = lambda h: None
    sys.modules["antenv.axon_hooks"] = mod
    bu.upload_artifacts = lambda tmpdir: f"local:{tmpdir}"


def kernel(_trace=False, _tmpdir=None, **inputs):
    global last_results
    if _trace:
        _enable_axon_trace()
    x = np.asarray(inputs["x"], dtype=np.float32)
    xpad = np.zeros((B, C, H, XW), np.float32)
    xpad[:, :, :, 1:W + 1] = x
    xpad = np.ascontiguousarray(xpad.astype(ml_dtypes.bfloat16))
    e_host, aow, akw, w1t, w2t, gb, sm = _host_prep(inputs)

    if "nc" not in _CACHE:
        _CACHE["nc"] = build_program()
    nc = _CACHE["nc"]

    shared = {"ew": e_host, "aow": aow, "akw": akw, "w1t": w1t,
              "w2t": w2t, "gb": gb, "sm": sm}
    in_maps = []
    for core in range(N_CORES):
        m = dict(shared)
        m["x"] = xpad[core * B_LOC:(core + 1) * B_LOC]
        in_maps.append(m)

    res = run_bass_kernel_spmd(nc, in_maps, list(range(N_CORES)),
                               trace=_trace, tmpdir=_tmpdir)
    last_results = res
    out = np.concatenate(
        [np.asarray(res.results[i]["out"]).astype(np.float32)
         for i in range(N_CORES)], axis=0)
    return out


# revision 29
# speedup vs baseline: 1.1181x; 1.1181x over previous
"""Trainium2 Bass kernel for the DEAM dense-CNN block.

Data-parallel over batch: 16 samples -> 8 cores x 2 samples.
Per sample: attention chain (GAP -> conv1d -> sigmoid/softmax heads),
dynamic per-sample 3x3 conv as 9 shifted matmuls, LGA gating branch,
fused add + batch BN (cross-core AllReduce of per-channel sums) + ReLU.

Channel shuffle is folded into host-side weight permutations plus a
permuted output DMA, so no on-chip data movement is spent on it.

v7 (trace-driven; lineage 294us -> 275 -> 254 -> ~this):
- chain split: softmax/kia path (part a) emitted and scheduled ahead
  of the LGA mlp path (part b); the 4.4us max-reduces are split in
  half and dependency-pinned off the critical path.
- GAP sums: tile-0 on ACT (Copy+accum_out) parallel with tile-1 on
  DVE; full-width dense reads (zero padding keeps sums exact, channel
  maxes verified positive).
- agg weight combine via 4x-mode tensor_scalar + 2x-mode tensor_tensor.
- BN: 2 stat AllReduces (ct0 mid-conv fully hidden; one final ct1) +
  a start-of-kernel alignment AllReduce; stat DMAs on gpsimd ring,
  readbacks on sync ring.
- z tiles and output bf16 (host upcasts); finalize full 128-partition
  tiles split across ACT and DVE.
"""

import numpy as np
import ml_dtypes

import concourse.bass as bass
import concourse.mybir as mybir
import concourse.tile as tile
from concourse import bacc
from concourse.bass_utils import run_bass_kernel_spmd
from concourse.masks import make_identity

F32 = mybir.dt.float32
BF16 = mybir.dt.bfloat16
AX = mybir.AxisListType
ALU = mybir.AluOpType
ACT = mybir.ActivationFunctionType

B, C, H, W = 16, 256, 64, 64
HW = H * W
KNUM, KS = 4, 3
N_CORES = 8
B_LOC = B // N_CORES          # 2 samples per core
NT = C // 128                 # 2 channel tiles
BN_EPS = 1e-5
XW = W + 2                    # padded row width 66
PQ = KS * KS                  # 9
EFREE = PQ * C                # 2304 free elems of an agg/E tile
EHALF = EFREE // 2            # 1152

REPLICAS = [list(range(N_CORES))]

# shift order: (0,0) first so the start=True matmul covers the full bank
SHIFTS = [(0, 0), (0, -1), (0, 1), (-1, -1), (-1, 0), (-1, 1),
          (1, -1), (1, 0), (1, 1)]


def _pin(inst, after, why):
    if after is not None:
        bass._add_dep_helper(inst.ins, after.ins, sync=False, reason=why)
    return inst


def build_program():
    cdt = BF16
    edt = BF16

    nc = bacc.Bacc("TRN2", target_bir_lowering=False, debug=False,
                   num_devices=N_CORES)

    x_d = nc.dram_tensor("x", [B_LOC, C, H, XW], cdt, kind="ExternalInput")
    e_d = nc.dram_tensor("ew", [KNUM, NT, 128, EFREE], edt,
                         kind="ExternalInput")
    aow_d = nc.dram_tensor("aow", [NT, 128, C], F32, kind="ExternalInput")
    akw_d = nc.dram_tensor("akw", [NT, 128, KNUM], F32, kind="ExternalInput")
    w1t_d = nc.dram_tensor("w1t", [128, 16], F32, kind="ExternalInput")
    w2t_d = nc.dram_tensor("w2t", [16, 128], F32, kind="ExternalInput")
    gb_d = nc.dram_tensor("gb", [NT, 128, 2], F32, kind="ExternalInput")
    sm_d = nc.dram_tensor("sm", [1, 8], F32, kind="ExternalInput")
    out_d = nc.dram_tensor("out", [B_LOC, C, H, W], BF16,
                           kind="ExternalOutput")
    dbg_d = nc.dram_tensor("dbg", [1, 8], F32, kind="ExternalOutput")

    with tile.TileContext(nc) as tc:
        with (
            tc.tile_pool(name="singles", bufs=1) as singles,
            tc.tile_pool(name="xq", bufs=1) as xq_pool,
            tc.tile_pool(name="z", bufs=1) as z_pool,
            tc.tile_pool(name="ep", bufs=16) as e_pool,
            tc.tile_pool(name="aggb", bufs=1) as aggb_pool,
            tc.tile_pool(name="atmp", bufs=2) as atmp_pool,
            tc.tile_pool(name="sp", bufs=2) as sp,
            tc.tile_pool(name="ob", bufs=2) as ob_pool,
            tc.tile_pool(name="psc", bufs=6, space="PSUM") as ps_conv,
            tc.tile_pool(name="pss", bufs=2, space="PSUM") as pss,
            tc.tile_pool(name="dram", bufs=8, space="DRAM") as dram,
        ):
            # ---- alignment AllReduce: absorbs cross-core launch skew ----
            alz = singles.tile([1, 8], F32, tag="alz")
            nc.vector.memset(alz[:, :], 0.0)
            ali = dram.tile([1, 8], F32, tag="ali", name="ali", bufs=1)
            alo = dram.tile([1, 8], F32, tag="alo", name="alo", bufs=1)
            nc.gpsimd.dma_start(out=ali[:, :], in_=alz[:, :])
            nc.gpsimd.collective_compute(
                "AllReduce", ALU.add, replica_groups=REPLICAS,
                ins=[ali[:, :].opt()], outs=[alo[:, :].opt()])
            albk = singles.tile([1, 8], F32, tag="albk")
            nc.gpsimd.dma_start(out=albk[:, :], in_=alo[:, :])
            nc.sync.dma_start(out=dbg_d[:, :], in_=albk[:, :])

            # ---- x[0] first (chain-0 gates everything), in row-halves
            # so the GAP sums can start before the full tile lands ----
            xq = [[None] * NT for _ in range(B_LOC)]
            for t in range(NT):
                xt = xq_pool.tile([128, H, XW], cdt, tag=f"xq0{t}",
                                  name=f"xq0{t}", bufs=1)
                for hf in range(2):
                    nc.sync.dma_start(
                        out=xt[:, hf * 32:(hf + 1) * 32, :],
                        in_=x_d[0, t * 128:(t + 1) * 128,
                                hf * 32:(hf + 1) * 32])
                xq[0][t] = xt

            # ---- constants ----
            ident = singles.tile([128, 128], F32, tag="ident")
            make_identity(nc, ident[:, :])
            smalls = singles.tile([1, 8], F32, tag="smalls")
            nc.sync.dma_start(out=smalls[:, :], in_=sm_d[:, :])
            aow_s = []
            akw_s = []
            for t in range(NT):
                a = singles.tile([128, C], F32, tag=f"aow{t}")
                nc.sync.dma_start(out=a[:, :], in_=aow_d[t])
                aow_s.append(a)
                k = singles.tile([128, KNUM], F32, tag=f"akw{t}")
                nc.sync.dma_start(out=k[:, :], in_=akw_d[t])
                akw_s.append(k)
            eps_t = singles.tile([128, 1], F32, tag="eps_t")
            nc.vector.memset(eps_t[:, :], BN_EPS)
            ones1 = singles.tile([1, 128], F32, tag="ones1")
            nc.vector.memset(ones1[:, :], 1.0)

            # per-ct batch-norm stat chunks (both samples) + AR results
            st = [singles.tile([128, 16, 6], F32, tag=f"st{t}",
                               name=f"st{t}") for t in range(NT)]
            sg0 = singles.tile([128, 2], F32, tag="sg0")
            sg1 = singles.tile([128, 2], F32, tag="sg1")
            ascr = [singles.tile([128, H * XW], BF16, tag=f"ascr{i}",
                                 name=f"ascr{i}") for i in range(2)]

            # ---- E weights, then x[1] ----
            e_tiles = {}
            for t in range(NT):
                for k in range(KNUM):
                    for hh in range(2):
                        et = e_pool.tile([128, EHALF], edt, tag="e")
                        nc.sync.dma_start(
                            out=et[:, :],
                            in_=e_d[k, t, :, hh * EHALF:(hh + 1) * EHALF])
                        e_tiles[(t, k, hh)] = et

            for t in range(NT):
                xt = xq_pool.tile([128, H, XW], cdt, tag=f"xq1{t}",
                                  name=f"xq1{t}", bufs=1)
                nc.sync.dma_start(out=xt[:, :, :],
                                  in_=x_d[1, t * 128:(t + 1) * 128])
                xq[1][t] = xt

            # weights not needed until the LGA mlp / BN finalize
            w1t_s = singles.tile([128, 16], F32, tag="w1t")
            nc.sync.dma_start(out=w1t_s[:, :], in_=w1t_d[:, :])
            w2t_s = singles.tile([16, 128], F32, tag="w2t")
            nc.sync.dma_start(out=w2t_s[:, :], in_=w2t_d[:, :])
            gb_s = singles.tile([128, NT, 2], F32, tag="gb")
            for t in range(NT):
                nc.sync.dma_start(out=gb_s[:, t, :], in_=gb_d[t])

            # ---- agg + z allocation ----
            aggb = [[None] * NT for _ in range(B_LOC)]
            for i in range(B_LOC):
                for t in range(NT):
                    aggb[i][t] = aggb_pool.tile(
                        [128, EFREE], cdt, tag=f"aggb{i}{t}",
                        name=f"aggb{i}{t}", bufs=1)
            z = [[None] * NT for _ in range(B_LOC)]
            for i in range(B_LOC):
                for t in range(NT):
                    z[i][t] = z_pool.tile([128, HW], BF16, tag=f"z{i}{t}",
                                          name=f"z{i}{t}", bufs=1)

            # ---- chain part a: GAP -> conv1d -> sigmoid/softmax -> kia ----
            def emit_chain_a(i, dve_after=None, act_after=None,
                             split=False):
                ch = {}
                gsum = sp.tile([128, NT], F32, tag="gsum",
                               name=f"gsum{i}", bufs=1)
                if split:
                    # per-half partial sums start as soon as each x DMA
                    # half lands (ACT for tile 0, DVE for tile 1)
                    hp = sp.tile([128, 4], F32, tag="hp")
                    for hf in range(2):
                        sl = slice(hf * H * XW // 2, (hf + 1) * H * XW // 2)
                        nc.scalar.activation(
                            out=ascr[i][:, sl],
                            in_=xq[i][0][:, :, :].rearrange(
                                "p h w -> p (h w)")[:, sl],
                            func=ACT.Copy, accum_out=hp[:, hf:hf + 1])
                        nc.vector.tensor_reduce(
                            hp[:, 2 + hf:3 + hf],
                            xq[i][1][:, :, :].rearrange(
                                "p h w -> p (h w)")[:, sl],
                            axis=AX.X, op=ALU.add)
                    nc.vector.tensor_add(gsum[:, 0:1], hp[:, 0:1],
                                         hp[:, 1:2])
                    nc.vector.tensor_add(gsum[:, 1:2], hp[:, 2:3],
                                         hp[:, 3:4])
                else:
                    _pin(nc.scalar.activation(
                        out=ascr[i][:, :],
                        in_=xq[i][0][:, :, :].rearrange("p h w -> p (h w)"),
                        func=ACT.Copy, accum_out=gsum[:, 0:1]),
                        act_after, "chain act-sum ordered")
                    _pin(nc.vector.tensor_reduce(
                        gsum[:, 1:2],
                        xq[i][1][:, :, :].rearrange("p h w -> p (h w)"),
                        axis=AX.X, op=ALU.add), dve_after,
                        "chain sum ordered")

                # gap -> free layout (two (128,1) -> (1,128) transposes)
                gf = []
                for t in range(NT):
                    g_ps = pss.tile([1, 128], F32, tag="pst")
                    nc.tensor.transpose(g_ps[:, :], gsum[:, t:t + 1],
                                        ident[:, :])
                    gf.append(g_ps)
                g2 = sp.tile([1, C + 2], F32, tag="g2")
                nc.vector.memset(g2[:, :], 0.0)
                for t in range(NT):
                    nc.vector.tensor_copy(
                        out=g2[0:1, 1 + t * 128:1 + (t + 1) * 128],
                        in_=gf[t][0:1, :])
                gs = sp.tile([1, 130], F32, tag="gs")
                nc.vector.memset(gs[:, :], 0.0)
                nc.vector.tensor_copy(out=gs[0:1, 1:129], in_=gf[0][0:1, :])

                # t = conv1d(gap_mean, att_w) : weights pre-scaled by 1/HW
                ta = sp.tile([1, C], F32, tag="ta")
                tb = sp.tile([1, C], F32, tag="tb")
                t_t = sp.tile([1, C], F32, tag="t_t")
                nc.vector.tensor_scalar_mul(ta[:, :], g2[0:1, 0:C],
                                            smalls[0:1, 0:1])
                nc.vector.scalar_tensor_tensor(tb[:, :], g2[0:1, 1:C + 1],
                                               smalls[0:1, 1:2], ta[:, :],
                                               ALU.mult, ALU.add)
                nc.vector.scalar_tensor_tensor(t_t[:, :], g2[0:1, 2:C + 2],
                                               smalls[0:1, 2:3], tb[:, :],
                                               ALU.mult, ALU.add)

                # s = conv1d(gap1_mean, lga_w) + b (bias in free layout)
                sa_ = sp.tile([1, 128], F32, tag="sa_")
                sb_ = sp.tile([1, 128], F32, tag="sb_")
                s_t = sp.tile([1, 128], F32, tag="s_t")
                nc.vector.tensor_scalar_mul(sa_[:, :], gs[0:1, 0:128],
                                            smalls[0:1, 3:4])
                nc.vector.scalar_tensor_tensor(sb_[:, :], gs[0:1, 1:129],
                                               smalls[0:1, 4:5], sa_[:, :],
                                               ALU.mult, ALU.add)
                nc.vector.scalar_tensor_tensor(s_t[:, :], gs[0:1, 2:130],
                                               smalls[0:1, 5:6], sb_[:, :],
                                               ALU.mult, ALU.add)
                nc.vector.tensor_scalar_add(s_t[:, :], s_t[:, :],
                                            smalls[0:1, 6:7])

                # transposes back to partition layout
                tps = sp.tile([128, NT], F32, tag="tps")
                ia = sp.tile([128, NT], F32, tag="ia")
                for t in range(NT):
                    tp_ps = pss.tile([128, 1], F32, tag="pst")
                    nc.tensor.transpose(tp_ps[:, :],
                                        t_t[0:1, t * 128:(t + 1) * 128],
                                        ident[0:1, 0:1])
                    nc.vector.tensor_copy(out=tps[:, t:t + 1],
                                          in_=tp_ps[:, :])
                    nc.scalar.activation(out=ia[:, t:t + 1], in_=tp_ps[:, :],
                                         func=ACT.Sigmoid)
                sk = sp.tile([128, 2], F32, tag="sk", name=f"sk{i}", bufs=1)
                sp_ps = pss.tile([128, 1], F32, tag="pst")
                nc.tensor.transpose(sp_ps[:, :], s_t[0:1, :],
                                    ident[0:1, 0:1])
                nc.scalar.activation(out=sk[:, 0:1], in_=sp_ps[:, :],
                                     func=ACT.Sigmoid)

                # kernel attention logits -> softmax (critical path to agg)
                kl_ps = pss.tile([KNUM, 1], F32, tag="pst")
                for t in range(NT):
                    nc.tensor.matmul(kl_ps[:, :], akw_s[t][:, :],
                                     tps[:, t:t + 1],
                                     start=(t == 0), stop=(t == NT - 1))
                kls = sp.tile([KNUM, 1], F32, tag="kls")
                nc.vector.tensor_copy(out=kls[:, :], in_=kl_ps[:, :])
                kt_ps = pss.tile([1, KNUM], F32, tag="pst")
                nc.tensor.transpose(kt_ps[:, :], kls[:, :],
                                    ident[0:KNUM, 0:KNUM])
                mx = sp.tile([1, 1], F32, tag="mx")
                nc.vector.reduce_max(mx[:, :], kt_ps[0:1, :], axis=AX.X)
                ex = sp.tile([1, KNUM], F32, tag="ex")
                nc.vector.tensor_scalar(out=ex[:, :], in0=kt_ps[0:1, :],
                                        scalar1=mx[:, :], scalar2=None,
                                        op0=ALU.subtract)
                exs = sp.tile([1, KNUM], F32, tag="exs")
                ch["exp_inst"] = nc.scalar.activation(out=exs[:, :],
                                                      in_=ex[:, :],
                                                      func=ACT.Exp)
                sm1 = sp.tile([1, 1], F32, tag="sm1")
                nc.vector.reduce_sum(sm1[:, :], exs[:, :], axis=AX.X)
                nc.vector.reciprocal(out=sm1[:, :], in_=sm1[:, :])
                katt = sp.tile([1, KNUM], F32, tag="katt")
                nc.vector.tensor_scalar_mul(katt[:, :], exs[:, :],
                                            sm1[:, :])
                kattb = sp.tile([128, KNUM], F32, tag="kattb")
                kb_ps = pss.tile([128, KNUM], F32, tag="pst")
                nc.tensor.matmul(kb_ps[:, :], ones1[:, :], katt[0:1, :],
                                 start=True, stop=True)
                nc.vector.tensor_copy(out=kattb[:, :], in_=kb_ps[:, :])

                # fold in_att into softmax scalars: kia[ci,k] = katt_k*ia[ci]
                kia = sp.tile([128, NT, KNUM], F32, tag="kia")
                kia_last = None
                for t in range(NT):
                    kia_last = nc.vector.tensor_scalar_mul(
                        kia[:, t, :], kattb[:, :], ia[:, t:t + 1])
                ch["kia_last"] = kia_last

                # out_att (permuted) per co tile -- needed only at drains
                oatt = sp.tile([128, NT], F32, tag="oatt",
                               name=f"oatt{i}", bufs=1)
                for ct in range(NT):
                    o_ps = pss.tile([128, 1], F32, tag="pst")
                    for t in range(NT):
                        nc.tensor.matmul(
                            o_ps[:, :],
                            aow_s[t][:, ct * 128:(ct + 1) * 128],
                            tps[:, t:t + 1],
                            start=(t == 0), stop=(t == NT - 1))
                    nc.scalar.activation(out=oatt[:, ct:ct + 1],
                                         in_=o_ps[:, :], func=ACT.Sigmoid)

                ch["kia"] = kia
                ch["oatt"] = oatt
                ch["sk"] = sk
                ch["gsum"] = gsum
                return ch

            # ---- chain part b: LGA mlp (max/mean -> mlp -> sk[1]) ----
            def emit_chain_b(i, ch, dve_after=None):
                vv = sp.tile([128, 2], F32, tag="vv")
                xf1 = xq[i][1][:, :, :].rearrange("p h w -> p (h w)")
                hx = sp.tile([128, 2], F32, tag="hx")
                _pin(nc.vector.tensor_reduce(
                    hx[:, 0:1], xf1[:, 0:H * XW // 2],
                    axis=AX.X, op=ALU.max), dve_after, "max half ordered")
                _pin(nc.vector.tensor_reduce(
                    hx[:, 1:2], xf1[:, H * XW // 2:],
                    axis=AX.X, op=ALU.max), dve_after, "max half ordered")
                nc.vector.tensor_reduce(vv[:, 0:1], hx[:, :],
                                        axis=AX.X, op=ALU.max)
                nc.vector.tensor_scalar_mul(vv[:, 1:2],
                                            ch["gsum"][:, 1:2], 1.0 / HW)
                h_ps = pss.tile([16, 2], F32, tag="pst")
                nc.tensor.matmul(h_ps[:, :], w1t_s[:, :], vv[:, :],
                                 start=True, stop=True)
                h_s = sp.tile([16, 2], F32, tag="h_s")
                nc.vector.tensor_scalar_max(h_s[:, :], h_ps[:, :], 0.0)
                m_ps = pss.tile([128, 2], F32, tag="pst")
                nc.tensor.matmul(m_ps[:, :], w2t_s[:, :], h_s[:, :],
                                 start=True, stop=True)
                mcp = sp.tile([128, 2], F32, tag="mcp")
                nc.vector.tensor_copy(out=mcp[:, :], in_=m_ps[:, :])
                chadd = sp.tile([128, 1], F32, tag="chadd")
                nc.vector.tensor_add(chadd[:, :], mcp[:, 0:1], mcp[:, 1:2])
                nc.scalar.activation(out=ch["sk"][:, 1:2], in_=chadd[:, :],
                                     func=ACT.Sigmoid)

            def emit_agg(i, ch):
                # dst = sum_k kia_k * E_k, via 4x-mode TS + 2x-mode TT add
                last = None
                for t in range(NT):
                    for hh in range(2):
                        dst = aggb[i][t][:, hh * EHALF:(hh + 1) * EHALF]
                        kap0 = ch["kia"][:, t, 0:1]
                        nc.vector.tensor_scalar_mul(dst, e_tiles[(t, 0, hh)],
                                                    kap0)
                        for k in range(1, KNUM):
                            tmp = atmp_pool.tile([128, EHALF], BF16,
                                                 tag="atmp")
                            nc.vector.tensor_scalar_mul(
                                tmp[:, :], e_tiles[(t, k, hh)],
                                ch["kia"][:, t, k:k + 1])
                            last = nc.vector.tensor_add(dst, dst, tmp[:, :])
                return last

            def emit_zinit(i, t, ch, after=None):
                return _pin(nc.vector.tensor_scalar_mul(
                    z[i][t][:, :], xq[i][t][:, :, 1:W + 1],
                    ch["sk"][:, t:t + 1]), after, "zinit ordered")

            def emit_conv(i, ct, ch):
                for grp in range(2):
                    banks = []
                    for j in range(4):
                        banks.append(ps_conv.tile([128, 8, W], F32,
                                                  tag="cps",
                                                  name=f"cps{j}"))
                    for cit in range(NT):
                        for (dp, dq) in SHIFTS:
                            pq = (dp + 1) * 3 + (dq + 1)
                            lo = pq * C + ct * 128
                            lhs = aggb[i][cit][:, lo:lo + 128]
                            for j in range(4):
                                chunk = grp * 4 + j
                                y0 = chunk * 8
                                ylo = max(y0, -dp)
                                yhi = min(y0 + 7, H - 1 - dp)
                                n_r = yhi - ylo + 1
                                if n_r <= 0:
                                    continue
                                out_ap = banks[j][:, ylo - y0:
                                                  ylo - y0 + n_r, :]
                                in_ap = xq[i][cit][:, ylo + dp:
                                                   ylo + dp + n_r,
                                                   1 + dq:1 + dq + W]
                                first = (cit == 0 and dp == 0 and dq == 0)
                                last = (cit == NT - 1
                                        and (dp, dq) == SHIFTS[-1])
                                nc.tensor.matmul(out_ap, lhs, in_ap,
                                                 start=first, stop=last,
                                                 skip_group_check=True)
                    # drain: z = psum * out_att + z ; then partial BN stats
                    for j in range(4):
                        chunk = grp * 4 + j
                        zsl = z[i][ct][:, chunk * 512:(chunk + 1) * 512]
                        nc.vector.scalar_tensor_tensor(
                            zsl, banks[j][:, :, :],
                            ch["oatt"][:, ct:ct + 1], zsl,
                            ALU.mult, ALU.add)
                        nc.vector.bn_stats(
                            out=st[ct][:, i * 8 + grp * 4 + j, :], in_=zsl)

            def emit_ar(name, sg_tile, ct):
                mv = sp.tile([128, 2], F32, tag="mv")
                nc.vector.bn_aggr(out=mv[:, :], in_=st[ct][:, :, :])
                m2 = sp.tile([128, 1], F32, tag="m2")
                nc.vector.tensor_mul(m2[:, :], mv[:, 0:1], mv[:, 0:1])
                sa = sp.tile([128, 2], F32, tag=f"sa{name}",
                             name=f"sa{name}")
                nc.vector.tensor_scalar_mul(sa[:, 0:1], mv[:, 0:1],
                                            float(B_LOC * HW))
                ex2 = sp.tile([128, 1], F32, tag="ex2")
                nc.vector.tensor_add(ex2[:, :], mv[:, 1:2], m2[:, :])
                nc.vector.tensor_scalar_mul(sa[:, 1:2], ex2[:, :],
                                            float(B_LOC * HW))
                si = dram.tile([128, 2], F32, tag=f"si{name}",
                               name=f"si{name}", bufs=1)
                so = dram.tile([128, 2], F32, tag=f"so{name}",
                               name=f"so{name}", bufs=1)
                nc.sync.dma_start(out=si[:, :], in_=sa[:, :])
                nc.gpsimd.collective_compute(
                    "AllReduce", ALU.add, replica_groups=REPLICAS,
                    ins=[si[:, :].opt()], outs=[so[:, :].opt()])
                nc.sync.dma_start(out=sg_tile[:, :], in_=so[:, :])

            out_view = out_d[:, :, :, :].rearrange(
                "b (cl cr) h w -> b cr cl (h w)", cr=4)
            n_total = float(B * HW)

            def emit_fin(t, sg_tile):
                tot = sp.tile([128, 2], F32, tag="tot")
                nc.vector.tensor_scalar_mul(tot[:, :], sg_tile[:, 0:2],
                                            1.0 / n_total)
                mean = tot[:, 0:1]
                ex2g = tot[:, 1:2]
                m2g = sp.tile([128, 1], F32, tag="m2g")
                nc.vector.tensor_mul(m2g[:, :], mean, mean)
                var = sp.tile([128, 1], F32, tag="var")
                nc.vector.tensor_sub(var[:, :], ex2g, m2g[:, :])
                rstd = sp.tile([128, 1], F32, tag="rstd")
                nc.scalar.activation(out=rstd[:, :], in_=var[:, :],
                                     func=ACT.Sqrt, bias=eps_t[:, :])
                nc.vector.reciprocal(out=rstd[:, :], in_=rstd[:, :])
                scl = sp.tile([128, 1], F32, tag="scl")
                nc.vector.tensor_mul(scl[:, :], gb_s[:, t, 0:1], rstd[:, :])
                tmpb = sp.tile([128, 1], F32, tag="tmpb")
                nc.vector.tensor_mul(tmpb[:, :], mean, scl[:, :])
                bia = sp.tile([128, 1], F32, tag="bia")
                nc.vector.tensor_sub(bia[:, :], gb_s[:, t, 1:2], tmpb[:, :])
                for i in range(B_LOC):
                    ob = ob_pool.tile([128, HW], BF16, tag="ob")
                    if i == 0:
                        nc.scalar.activation(out=ob[:, :], in_=z[i][t][:, :],
                                             func=ACT.Relu, bias=bia[:, :],
                                             scale=scl[:, :])
                    else:
                        nc.vector.tensor_scalar(
                            out=ob[:, :], in0=z[i][t][:, :],
                            scalar1=scl[:, :], scalar2=bia[:, :],
                            op0=ALU.mult, op1=ALU.add)
                        nc.vector.tensor_scalar_max(ob[:, :], ob[:, :], 0.0)
                    for ph in range(2):
                        nc.sync.dma_start(
                            out=out_view[i, 2 * t + ph, :, :],
                            in_=ob[ph * 64:(ph + 1) * 64, :])

            # ---- program flow ----
            ch0 = emit_chain_a(0, split=True)
            agg0_last = emit_agg(0, ch0)
            zi00 = emit_zinit(0, 0, ch0, after=ch0["kia_last"])
            emit_chain_b(0, ch0, dve_after=agg0_last)
            zi01 = emit_zinit(0, 1, ch0)
            emit_conv(0, 0, ch0)
            ch1 = emit_chain_a(1, dve_after=zi01,
                               act_after=ch0["exp_inst"])
            emit_conv(0, 1, ch0)
            agg1_last = emit_agg(1, ch1)
            zi10 = emit_zinit(1, 0, ch1)
            emit_conv(1, 0, ch1)
            emit_chain_b(1, ch1, dve_after=agg1_last)
            emit_zinit(1, 1, ch1)
            emit_ar("0", sg0, 0)
            emit_conv(1, 1, ch1)
            emit_ar("1", sg1, 1)
            emit_fin(0, sg0)
            emit_fin(1, sg1)
    nc.finalize()
    return nc


def _host_prep(inputs):
    """Numpy-side weight re-layouts (all small except ede transpose)."""
    c = np.arange(C)
    pinv = (c % 64) * 4 + c // 64          # output-channel permutation
    ede = np.ascontiguousarray(inputs["ede_weight"], dtype=np.float32)
    ede_p = ede[:, pinv]                    # permute co axis
    # -> [k, ci, pq, co] so an SBUF agg tile is [ci_part, pq*256+co]
    e_host = np.ascontiguousarray(
        ede_p.transpose(0, 2, 3, 4, 1).reshape(KNUM, NT, 128, EFREE))
    e_host = e_host.astype(ml_dtypes.bfloat16)
    aow = np.ascontiguousarray(
        inputs["att_out_w"][pinv].T.reshape(NT, 128, C), dtype=np.float32)
    akw = np.ascontiguousarray(
        inputs["att_kernel_w"].T.reshape(NT, 128, KNUM), dtype=np.float32)
    w1t = np.ascontiguousarray(inputs["lga_mlp_w1"].T, dtype=np.float32)
    w2t = np.ascontiguousarray(inputs["lga_mlp_w2"].T, dtype=np.float32)
    gb = np.stack([np.asarray(inputs["bn_gamma"])[pinv].reshape(NT, 128),
                   np.asarray(inputs["bn_beta"])[pinv].reshape(NT, 128)],
                  axis=-1).astype(np.float32)
    aw = np.asarray(inputs["att_conv1d_w"], dtype=np.float32) / HW
    lw = np.asarray(inputs["lga_conv1d_w"], dtype=np.float32) / HW
    lb = float(np.asarray(inputs["lga_conv1d_b"]).reshape(-1)[0])
    sm = np.array([[aw[0], aw[1], aw[2], lw[0], lw[1], lw[2], lb, 0.0]],
                  dtype=np.float32)
    return e_host, aow, akw, w1t, w2t, gb, sm


_CACHE = {}
last_results = None


def _enable_axon_trace():
    """Register the NTFF profile hook that the agent image leaves out."""
    import sys
    import types

    import concourse.bass_utils as bu
    if "antenv.axon_hooks" in sys.modules:
        return
    from trn_agent_boot.trn_boot import _ntff_profile_via_ctypes
    hook = _ntff_profile_via_ctypes("/opt/axon/libaxon_pjrt.so")
    mod = types.ModuleType("antenv.axon_hooks")
    mod.get_axon_ntff_profile_hook = lambda: hook
    mod.set_axon_ntff_profile_hook = lambda h: None
    sys.modules["antenv.axon_hooks"] = mod
    bu.upload_artifacts = lambda tmpdir: f"local:{tmpdir}"


def kernel(_trace=False, _tmpdir=None, **inputs):
    global last_results
    if _trace:
        _enable_axon_trace()
    x = np.asarray(inputs["x"], dtype=np.float32)
    xpad = np.zeros((B, C, H, XW), np.float32)
    xpad[:, :, :, 1:W + 1] = x
    xpad = np.ascontiguousarray(xpad.astype(ml_dtypes.bfloat16))
    e_host, aow, akw, w1t, w2t, gb, sm = _host_prep(inputs)

    if "nc" not in _CACHE:
        _CACHE["nc"] = build_program()
    nc = _CACHE["nc"]

    shared = {"ew": e_host, "aow": aow, "akw": akw, "w1t": w1t,
              "w2t": w2t, "gb": gb, "sm": sm}
    in_maps = []
    for core in range(N_CORES):
        m = dict(shared)
        m["x"] = xpad[core * B_LOC:(core + 1) * B_LOC]
        in_maps.append(m)

    res = run_bass_kernel_spmd(nc, in_maps, list(range(N_CORES)),
                               trace=_trace, tmpdir=_tmpdir)
    last_results = res
    out = np.concatenate(
        [np.asarray(res.results[i]["out"]).astype(np.float32)
         for i in range(N_CORES)], axis=0)
    return out
